# revision 49
# baseline (speedup 1.0000x reference)
"""Trainium2 Bass kernel for nn_Attention (B=4, N=2048, DIM=1024, 16 heads x 64).

Sharding: 8 cores = 4 batches x 2 head-groups. Core c handles batch c//2 and
heads [8*(c%2), 8*(c%2)+8). Each core computes QKV projection, attention and
the output projection for its (batch, head-group); the two cores sharing a
batch produce partial output projections that the host sums (+ bias).

Device-side layout (per core):
  xT   [128, 8, 2048]  bf16  x^T (contraction chunk k on dim1) - ONE tile so
                             each DMA slice is a single Sync-queue dispatch
                             (DMA dispatch costs ~0.6us on the queue; the
                             prologue is dispatch-bound otherwise)
  wq   [128, 8, 1536]  bf16  w_qkv chunks (Q|K|V cols)
  qT/kT [128, 2048] x4 bf16  Q^T / K^T per pair-group; partitions 0-63 even
                             head, 64-127 odd head
  vprime [2048, 520]  bf16  V with a ones column per head (65-stride groups)
  S^T: per j-tile two K=64 matmuls [128j, 512i] (one per head) that run
      CONCURRENTLY on the two 64x128 PE row-tiles, into a 2-bank PSUM tile
      [128, 1024]; psS bufs=3 decouples the S->exp chain
  P^T = exp(SCALE * S^T) on ACT ('A' tiles) or DVE Schraudolph ('D' tiles)
  AV: out[i, 0:65 | 65:130] = P^T.T @ [V | 1], accumulated over j-tiles in
      one PSUM bank, normalized per partition into a [128, 512] staging
      tile; ONE batched DMA-XBAR transpose per (pg, ic) -> attnT.
  proj^T = W_out.T-tiled matmuls -> DRAM [1024, 2048] fp16 partials

Q/K projections for pg+1 are interleaved as fillers in pg's combos; V' for
token tiles 12-15 is deferred into the first combo to shorten the prologue.
"""

import sys

if "/opt/trn_rl_repo" not in sys.path:
    sys.path.insert(0, "/opt/trn_rl_repo")

import numpy as np
import ml_dtypes

import concourse.bass as bass
import concourse.mybir as mybir
from concourse.tile import TileContext
from concourse.bass_utils import run_bass_kernel_spmd

P = 128
B, N, DIM = 4, 2048, 1024
HEADS, DIM_HEAD = 16, 64
INNER = HEADS * DIM_HEAD
SCALE = DIM_HEAD**-0.5
LOCAL_HEADS = 8          # heads per core
LOCAL_INNER = LOCAL_HEADS * DIM_HEAD      # 512
QKV_COLS = 3 * LOCAL_INNER                # 1536
TT = N // P              # 16 token tiles
KC = DIM // P            # 8 contraction chunks
IC = N // 512            # 4 query chunks of 512
JT = N // P              # 16 key tiles
PG = LOCAL_HEADS // 2    # 4 head pair-groups
NT = DIM // P            # 8 output col tiles
KT = LOCAL_INNER // P    # 4 proj contraction chunks

BF16 = mybir.dt.bfloat16
F32 = mybir.dt.float32
F16 = mybir.dt.float16
I16 = mybir.dt.int16
EXP = mybir.ActivationFunctionType.Exp
MULT = mybir.AluOpType.mult
ADD = mybir.AluOpType.add

# exp engine per j-tile within a combo: 'A' = ACT exact exp, 'D' = DVE
# Schraudolph (bf16-bitcast affine; rel err ~3% on those tiles).  Keeps ACT
# off the critical path; D off j-tiles 14/15 which gate the next combo's
# first S matmuls through the psS pool.
EXP_PATTERN = "AAAADAAADAAADAAA"
EXPC = 5.0
EXPA = float(SCALE * np.log2(np.e) * 128.0)
EXPB = float(127 * 128) - EXPC + 0.5


def _split_multi_waits(nc):
    """This env's walrus encodes at most ONE sync-wait per instruction (and
    ZERO on DMA-transpose); hoist extras onto InstEventSemaphore carriers
    inserted just before, same engine."""
    f = nc.m.functions[0]
    rebuilt = []
    for blk in f.blocks:
        newlist = []
        for inst in blk.instructions:
            si = inst.sync_info
            keep = 0 if isinstance(inst, mybir.InstDmaTransposeAnt) else 1
            if si is not None and len(si.on_wait) > keep:
                waits = list(si.on_wait)
                eng = inst.engine
                hoist = waits if keep == 0 else waits[:-1]
                for w in hoist:
                    ev = mybir.InstEventSemaphore(
                        name=nc.get_next_instruction_name(), ins=[], outs=[])
                    ev.engine = eng
                    ev.sync_info = mybir.SyncInfo(on_wait=[w], on_update=[])
                    newlist.append(ev)
                inst.sync_info = mybir.SyncInfo(
                    on_wait=waits[-1:] if keep else [],
                    on_update=list(si.on_update))
            newlist.append(inst)
        rebuilt.append((blk, newlist))
    for blk, newlist in rebuilt:
        blk.instructions = newlist
    return nc


def build_attention_nc(split_waits=True):
    nc = bass.Bass(trn_type="TRN2", num_devices=8)
    x_d = nc.dram_tensor("x", [DIM, N], BF16, kind="ExternalInput")
    wqkv_d = nc.dram_tensor("wqkv", [DIM, QKV_COLS], BF16, kind="ExternalInput")
    wout_d = nc.dram_tensor("wout", [LOCAL_INNER, DIM], BF16, kind="ExternalInput")
    o_d = nc.dram_tensor("o", [DIM, N], F16, kind="ExternalOutput")

    V0 = 2 * LOCAL_INNER  # start of V columns in wqkv
    x_src = x_d[:].rearrange("(k p) c -> p k c", p=P)
    wqkv_src = wqkv_d[:].rearrange("(k p) c -> p k c", p=P)
    wout_src = wout_d[:].rearrange("(k p) c -> p k c", p=P)

    with TileContext(nc, num_cores=8) as tc:
        with tc.tile_pool(name="persist", bufs=1) as persist:
            xT = persist.tile([P, KC, N], BF16, tag="xT")
            wq = persist.tile([P, KC, QKV_COLS], BF16, tag="wq")
            qT = [persist.tile([P, N], BF16, name=f"qT{g}", tag=f"qT{g}")
                  for g in range(KT)]
            kT = [persist.tile([P, N], BF16, name=f"kT{g}", tag=f"kT{g}")
                  for g in range(KT)]
            vprime = [persist.tile([P, 65 * LOCAL_HEADS], BF16, name=f"vp{t}",
                                   tag=f"vp{t}") for t in range(TT)]
            wout = persist.tile([P, KT, DIM], BF16, tag="wout")

            # ---- DMA: 7 coarse dispatches, split across the two HWDGE
            # queues (Sync + Scalar) so dispatch overhead never starves the
            # PE.  Earliest-needed bytes first. ----
            nc.sync.dma_start(wq[:, :, V0:QKV_COLS],
                              wqkv_src[:, :, V0:QKV_COLS])
            nc.scalar.dma_start(xT[:, :, 0:512], x_src[:, :, 0:512])
            nc.sync.dma_start(xT[:, :, 512:1024], x_src[:, :, 512:1024])
            nc.scalar.dma_start(wq[:, :, 0:V0], wqkv_src[:, :, 0:V0])
            nc.sync.dma_start(xT[:, :, 1024:1536], x_src[:, :, 1024:1536])
            nc.scalar.dma_start(xT[:, :, 1536:2048], x_src[:, :, 1536:2048])
            nc.sync.dma_start(wout[:], wout_src[:])
            for tt in range(TT):
                nc.gpsimd.memset(vprime[tt][:], 1.0)

            def v_run_in(pool, tt, tag="v"):
                ps = pool.tile([P, 512], F32, name=f"v_{tt}", tag=tag)
                for k in range(KC):
                    nc.tensor.matmul(
                        ps[:], lhsT=xT[:, k, tt * P:(tt + 1) * P],
                        rhs=wq[:, k, V0:QKV_COLS],
                        start=(k == 0), stop=(k == KC - 1))
                for h in range(LOCAL_HEADS):
                    nc.vector.tensor_copy(
                        vprime[tt][:, h * 65:h * 65 + 64],
                        ps[:, h * 64:(h + 1) * 64])

            def qk_run_in(pool, ct, ic, tag="v"):
                # ct 0..3 -> Q pair-group ct; ct 4..7 -> K pair-group ct-4
                ps = pool.tile([P, 512], F32, name=f"qk_{ct}_{ic}", tag=tag)
                for k in range(KC):
                    nc.tensor.matmul(
                        ps[:], lhsT=wq[:, k, ct * P:(ct + 1) * P],
                        rhs=xT[:, k, ic * 512:(ic + 1) * 512],
                        start=(k == 0), stop=(k == KC - 1))
                dst = qT[ct] if ct < KT else kT[ct - KT]
                nc.vector.tensor_copy(dst[:, ic * 512:(ic + 1) * 512], ps[:])

            # ---- Stage A+B: V' (token tiles 0-11; 12-15 deferred) and the
            # pair-0 Q/K projections, ordered by input arrival. ----
            with tc.tile_pool(name="psBv", bufs=4, space="PSUM") as psBv:
                for tt in range(0, 4):
                    v_run_in(psBv, tt)
                qk_run_in(psBv, KT, 0)        # K pair0, ic0
                for tt in range(4, 8):
                    v_run_in(psBv, tt)
                qk_run_in(psBv, KT, 1)
                qk_run_in(psBv, 0, 0)         # Q pair0, ic0
                for tt in range(8, 12):
                    v_run_in(psBv, tt)
                qk_run_in(psBv, KT, 2)
                qk_run_in(psBv, KT, 3)
                qk_run_in(psBv, 0, 1)

            # ---- Stages C+D: pipelined attention + projection ----
            with tc.tile_pool(name="pT", bufs=30) as pT_pool, \
                 tc.tile_pool(name="psS", bufs=3, space="PSUM") as psS, \
                 tc.tile_pool(name="psF", bufs=1, space="PSUM") as psF, \
                 tc.tile_pool(name="psAV", bufs=1, space="PSUM") as psAV, \
                 tc.tile_pool(name="smallsb", bufs=8) as smallsb, \
                 tc.tile_pool(name="attnst", bufs=3) as attnst, \
                 tc.tile_pool(name="osb", bufs=3) as osb_pool:
                attnT_t = {}
                for kt in range(KT):
                    for ic in range(IC):
                        attnT_t[(kt, ic)] = persist.tile(
                            [P, 512], BF16, name=f"attnT_{kt}_{ic}",
                            tag=f"attnT_{kt}_{ic}")

                def qk_run(ct, ic):
                    qk_run_in(psF, ct, ic, "f")

                def v_run(tt):
                    v_run_in(psF, tt, tag="f")

                def proj_run(nt, ic, pool=None, tag="f"):
                    ps = (pool or psF).tile([P, 512], F32,
                                            name=f"proj_{nt}_{ic}", tag=tag)
                    for kt in range(KT):
                        nc.tensor.matmul(
                            ps[:], lhsT=wout[:, kt, nt * P:(nt + 1) * P],
                            rhs=attnT_t[(kt, ic)][:],
                            start=(kt == 0), stop=(kt == KT - 1))
                    osb = osb_pool.tile([P, 512], F16, name=f"osb_{nt}_{ic}",
                                        tag="osb")
                    nc.vector.tensor_copy(osb[:], ps[:])
                    nc.sync.dma_start(
                        o_d[nt * P:(nt + 1) * P, ic * 512:(ic + 1) * 512], osb[:])

                def av_gen(pg, ic, pT):
                    # AV + normalize for one (pg, ic) in 8-MM quanta; one
                    # batched XBAR transpose into attnT at the end.
                    attn4 = attnst.tile([P, 512], BF16, tag="attn4")
                    for it in range(4):
                        av = psAV.tile([P, 130], F32, name=f"av_{pg}_{ic}_{it}",
                                       tag="av")
                        for half in range(2):
                            base = 512 * half
                            voff = (2 * pg + half) * 65
                            for j0 in range(0, JT, 8):
                                for jt in range(j0, j0 + 8):
                                    nc.tensor.matmul(
                                        av[:, 65 * half:65 * half + 65],
                                        lhsT=pT[jt][:, base + it * P:
                                                    base + (it + 1) * P],
                                        rhs=vprime[jt][:, voff:voff + 65],
                                        start=(jt == 0), stop=(jt == JT - 1))
                                yield
                        for half in range(2):
                            recip = smallsb.tile([P, 1], F32, tag="recip")
                            nc.vector.reciprocal(
                                recip[:], av[:, 65 * half + 64:65 * half + 65])
                            nc.vector.tensor_scalar_mul(
                                attn4[:, it * P + half * 64:
                                      it * P + (half + 1) * 64],
                                av[:, 65 * half:65 * half + 64], recip[:])
                        yield
                    nc.sync.dma_start_transpose(
                        attnT_t[(pg, ic)][:].rearrange("p (a b) -> p a b", b=P),
                        attn4[:])

                def advance(gen, n):
                    for _ in range(n):
                        if next(gen, "done") == "done":
                            return None
                    return gen

                backlog = None
                for pg in range(PG):
                    if pg + 1 < PG:
                        filler = [(qk_run, pg + 1, ic) for ic in range(IC)] + \
                                 [(qk_run, KT + pg + 1, ic) for ic in range(IC)]
                        if pg == 0:
                            # Q-pair0 ic2/3 + deferred V' token tiles 12-15;
                            # all must be EMITTED before the first av_gen
                            # advance (combo ic1) touches vprime[12..15].
                            filler = [(qk_run, 0, 2), (qk_run, 0, 3),
                                      (v_run, 12, None), (v_run, 13, None),
                                      (v_run, 14, None), (v_run, 15, None)] + \
                                     filler
                    else:
                        # last pg: output projections of completed ic slices
                        filler = [(proj_run, nt, ic) for ic in range(IC - 2)
                                  for nt in range(NT)]
                    for ic in range(IC):
                        # slots are per j-PAIR (8 pair positions per combo)
                        if pg == 0 and ic == 0:
                            slots = (1, 2, 3, 4, 5, 6)
                        elif pg == 0 and ic == 1:
                            slots = (1, 3, 5, 7)
                        elif pg + 1 < PG:
                            slots = (2, 5)
                        else:
                            slots = (0, 1, 2, 3, 4, 5, 6, 7)
                        i0 = ic * 512
                        pT = []
                        for jp in range(JT // 2):
                            pss = []
                            for jt in (2 * jp, 2 * jp + 1):
                                ps = psS.tile([P, 1024], F32)
                                nc.tensor.matmul(
                                    ps[:, 0:512],
                                    lhsT=kT[pg][0:64, jt * P:(jt + 1) * P],
                                    rhs=qT[pg][0:64, i0:i0 + 512])
                                nc.tensor.matmul(
                                    ps[:, 512:1024],
                                    lhsT=kT[pg][64:128, jt * P:(jt + 1) * P],
                                    rhs=qT[pg][64:128, i0:i0 + 512])
                                pss.append(ps)
                            for jt, ps in zip((2 * jp, 2 * jp + 1), pss):
                                pt = pT_pool.tile([P, 1024], BF16)
                                if EXP_PATTERN[jt] == "A":
                                    nc.scalar.activation(pt[:], ps[:], EXP,
                                                         scale=SCALE)
                                else:
                                    nc.vector.tensor_scalar(
                                        pt[:].bitcast(I16), ps[:], EXPA, EXPB,
                                        MULT, ADD)
                                pT.append(pt)
                            if backlog is not None:
                                backlog = advance(backlog,
                                                  3 if pg == PG - 1 else 2)
                            # pg3's proj fillers must not be emitted before
                            # av_gen(pg3, ic0/ic1) has written their attnT
                            if jp in slots and filler and (pg < PG - 1
                                                          or ic >= 2):
                                fn, a, b = filler.pop(0)
                                if b is None:
                                    fn(a)
                                else:
                                    fn(a, b)
                        while backlog is not None:
                            backlog = advance(backlog, 4)
                        backlog = av_gen(pg, ic, pT)
                tailq = [(nt, IC - 2) for nt in range(NT)]
                while backlog is not None:
                    backlog = advance(backlog, 3)
                    if tailq:
                        nt, ic2 = tailq.pop(0)
                        proj_run(nt, ic2)
                for nt, ic2 in tailq:
                    proj_run(nt, ic2)
                for nt in range(NT):
                    proj_run(nt, IC - 1)

    if split_waits:
        _split_multi_waits(nc)
    return nc


_NC_CACHE = {}


def _get_nc():
    if "nc" not in _NC_CACHE:
        _NC_CACHE["nc"] = build_attention_nc()
    return _NC_CACHE["nc"]


def make_in_maps(x, w_qkv, w_out):
    bf = ml_dtypes.bfloat16
    in_maps = []
    for c in range(8):
        b, g = c // 2, c % 2
        lo = LOCAL_INNER * g
        wq = w_qkv[:, lo:lo + LOCAL_INNER]
        wk = w_qkv[:, INNER + lo:INNER + lo + LOCAL_INNER]
        wv = w_qkv[:, 2 * INNER + lo:2 * INNER + lo + LOCAL_INNER]
        in_maps.append({
            "x": np.ascontiguousarray(x[b].T).astype(bf),
            "wqkv": np.ascontiguousarray(
                np.concatenate([wq, wk, wv], axis=1)).astype(bf),
            "wout": np.ascontiguousarray(
                w_out[lo:lo + LOCAL_INNER, :]).astype(bf),
        })
    return in_maps


def combine_outputs(results, b_out):
    out = np.empty((B, N, DIM), dtype=np.float32)
    for b in range(B):
        acc = (results[2 * b]["o"].astype(np.float32)
               + results[2 * b + 1]["o"].astype(np.float32))
        out[b] = acc.T + b_out[None, :]
    return out


def kernel(x, w_qkv, w_out, b_out, _trace=False):
    x = np.asarray(x, dtype=np.float32)
    w_qkv = np.asarray(w_qkv, dtype=np.float32)
    w_out = np.asarray(w_out, dtype=np.float32)
    b_out = np.asarray(b_out, dtype=np.float32)
    nc = _get_nc()
    in_maps = make_in_maps(x, w_qkv, w_out)
    res = run_bass_kernel_spmd(nc, in_maps, core_ids=list(range(8)), trace=_trace)
    out = combine_outputs(res.results, b_out)
    if _trace:
        return out, res
    return out
